# revision 2
# baseline (speedup 1.0000x reference)
"""TRN2 Bass kernel for nn_ClassSemantic (scatter_memory).

Strategy
--------
Data-parallel over batch: core k owns samples 4k..4k+3 and runs
projection (1x1 conv) + memory-gather attention + concat.

The problem is HBM-bandwidth bound (feats in + concat out dominate), so
all device I/O is bf16: the host casts feats/weights/queue rows to bf16
before staging, the device computes with fp32 PSUM accumulation, writes
the output as bf16, and the host upcasts to fp32.  This halves HBM
traffic vs fp32 (33.5 MB/core -> ~94 us roofline at 358 GB/s).

The sequential EMA queue update depends on the per-sample masked
feature means only, which are algebraically separable:
    feat_b = mean_hw((Wp@f + bp) * pred) = Wp @ mean_hw(f * pred) + bp * mean(pred)
The inner reduction (134 MFLOP, 0.4% of total work) is computed on the
host, then the exactly-sequential 32-step EMA scan (tiny: [4,20,256]
state) runs on the host in float64 and the final queue rows are shipped
to every core as constants.  The device never needs a collective.

Softmax over the 20 memory slots: logits are empirically in [-3, 3]
(queue rows are ~unit-norm, x ~ N(0,1)), so exp() without max
subtraction is safe.  Column sums / broadcasts across the 20-partition
axis are done with tiny ones-matmuls on the PE.

Output layout: one SBUF tile o_sb[128, 4, HW] per sample holds the
whole [512, HW] output (hh 0..1 = attention half, hh 2..3 = x half) so
each chunk needs a single store DMA on the ACT HWDGE ring; feats loads
ride the SP HWDGE ring.
"""
import os
import numpy as np
import ml_dtypes
from contextlib import ExitStack

B, IN_C, H, W_SP = 32, 512, 64, 64
CODE, CLASSES, MEM = 256, 4, 20
HW = H * W_SP              # 4096
NCORES = 8
BPC = B // NCORES          # 4 samples per core
DECAY, EPS = 0.9, 1e-12
NCH = 8                    # n-chunks per sample
NT = HW // NCH             # 512 spatial positions per chunk

BF16 = ml_dtypes.bfloat16

_PROGRAM_CACHE = {}
LAST_RESULTS = None        # stash for test harness introspection


def _host_queue_update(feats, preds, labels, flag, queue, Wp, bp):
    """Final queue after the reference's sequential EMA scan (float64)."""
    if int(flag) != 1:
        return queue.astype(np.float32)
    f3 = feats.reshape(B, IN_C, HW)
    p2 = preds.reshape(B, HW)
    # g_b = mean_n feats_b[:, n] * pred_b[n]  (batched sgemv)
    g = np.matmul(f3, p2[:, :, None])[:, :, 0] / np.float32(HW)
    feat = g @ Wp.T.astype(np.float32) + bp[None, :] * p2.mean(axis=1)[:, None]
    q = queue.astype(np.float64)
    for i in range(B):
        l = int(labels[i])
        f = feat[i].astype(np.float64)
        slot = q[l]
        logit = slot @ f
        upd = logit[:, None] * f[None, :]
        nrm = np.sqrt((upd * upd).sum(axis=1, keepdims=True))
        upd = upd / np.maximum(nrm, EPS)
        q[l] = DECAY * slot + (1.0 - DECAY) * upd
    return q.astype(np.float32)


def _build_program():
    from concourse import bacc, mybir
    import concourse.tile as tile

    f32, bf16 = mybir.dt.float32, mybir.dt.bfloat16
    nc = bacc.Bacc("TRN2", target_bir_lowering=False, debug=False)

    feats_in = nc.dram_tensor("feats", [BPC, IN_C, HW], bf16, kind="ExternalInput").ap()
    wpt_in = nc.dram_tensor("wpt", [IN_C, CODE], bf16, kind="ExternalInput").ap()
    bp_in = nc.dram_tensor("bpc", [128, 2], f32, kind="ExternalInput").ap()
    qat_in = nc.dram_tensor("qat", [BPC, CODE, MEM], bf16, kind="ExternalInput").ap()
    qa_in = nc.dram_tensor("qa", [BPC, MEM, CODE], bf16, kind="ExternalInput").ap()
    ones20_in = nc.dram_tensor("ones20", [MEM, MEM], bf16, kind="ExternalInput").ap()
    out_ext = nc.dram_tensor("out", [BPC, 2 * CODE, HW], bf16, kind="ExternalOutput").ap()

    with tile.TileContext(nc) as tc, ExitStack() as ctx:
        consts = ctx.enter_context(tc.tile_pool(name="consts", bufs=1))
        fpool = ctx.enter_context(tc.tile_pool(name="fpool", bufs=4))
        opool = ctx.enter_context(tc.tile_pool(name="opool", bufs=2))
        spool = ctx.enter_context(tc.tile_pool(name="spool", bufs=2))
        spool4 = ctx.enter_context(tc.tile_pool(name="spool4", bufs=5))
        ppp = ctx.enter_context(tc.tile_pool(name="ppp", bufs=2, space="PSUM"))
        pps = ctx.enter_context(tc.tile_pool(name="pps", bufs=2, space="PSUM"))
        ppc = ctx.enter_context(tc.tile_pool(name="ppc", bufs=2, space="PSUM"))
        ppu = ctx.enter_context(tc.tile_pool(name="ppu", bufs=2, space="PSUM"))

        # constants load on the scalar HWDGE ring so the sync ring starts
        # streaming feats immediately
        wpt_sb = consts.tile([128, 4, CODE], bf16, name="wpt_sb")       # [p, kchunk, o]
        nc.scalar.dma_start(wpt_sb[:], wpt_in.rearrange("(kk p) m -> p kk m", p=128))
        bp_sb = consts.tile([128, 2], f32, name="bp_sb")                # [p, half]
        nc.scalar.dma_start(bp_sb[:], bp_in[:])
        qat_sb = consts.tile([128, BPC, 2, MEM], bf16, name="qat_sb")   # [p, b, kchunk, m]
        qa_sb = consts.tile([MEM, BPC, CODE], bf16, name="qa_sb")       # [m, b, c]
        ones20_sb = consts.tile([MEM, MEM], bf16, name="ones20_sb")

        def load_attn_consts():
            nc.scalar.dma_start(qat_sb[:], qat_in.rearrange("b (kk p) m -> p b kk m", p=128))
            nc.scalar.dma_start(qa_sb[:], qa_in.rearrange("b m c -> m b c"))
            nc.scalar.dma_start(ones20_sb[:], ones20_in[:])

        o_tiles = {}
        pexp_t = {}
        cs_t = {}
        pn_t = {}
        T = BPC * NCH

        def bj(c):
            return c // NCH, c % NCH

        def proj_chunk(c):
            b, j = bj(c)
            if j == 0:
                # hh 0..1: attention half (channels 0..255); hh 2..3: x half
                o_tiles[b] = opool.tile([128, 4, HW], bf16, tag="o_sb", name=f"o_sb{b}")
            feats_b = feats_in[b].rearrange("(kk p) n -> p kk n", p=128)
            o_sb = o_tiles[b]
            ft = fpool.tile([128, 4, NT], bf16, tag="ft", name=f"ft{c}")
            nc.sync.dma_start(ft[:], feats_b[:, :, j * NT:(j + 1) * NT])
            for h in range(2):
                ps = ppp.tile([128, NT], f32, tag="proj_ps", name=f"pps{c}_{h}")
                for kk in range(4):
                    nc.tensor.matmul(
                        ps[:], wpt_sb[:, kk, h * 128:(h + 1) * 128], ft[:, kk, :],
                        start=(kk == 0), stop=(kk == 3))
                # psum -> sbuf with per-channel bias, rounding to bf16
                if h == 0:
                    nc.scalar.activation(
                        o_sb[:, 2 + h, j * NT:(j + 1) * NT], ps[:],
                        mybir.ActivationFunctionType.Identity,
                        bias=bp_sb[:, h:h + 1])
                else:
                    nc.vector.tensor_scalar_add(
                        o_sb[:, 2 + h, j * NT:(j + 1) * NT], ps[:], bp_sb[:, h:h + 1])

        def logit_stage(c):
            b, j = bj(c)
            o_sb = o_tiles[b]
            js = slice(j * NT, (j + 1) * NT)
            lg = pps.tile([MEM, NT], f32, tag="logit_ps", name=f"lg{c}")
            for kk in range(2):
                nc.tensor.matmul(lg[:], qat_sb[:, b, kk, :], o_sb[:, 2 + kk, js],
                                 start=(kk == 0), stop=(kk == 1))
            pexp = spool4.tile([MEM, NT], bf16, tag="pexp", name=f"pexp{c}")
            nc.scalar.activation(pexp[:], lg[:], mybir.ActivationFunctionType.Exp)
            pexp_t[c] = pexp

        def sum_stage(c):
            # lhsT = all-ones [20,20]: every output partition gets the
            # column sum, so no cross-partition broadcast is needed later.
            cs = ppc.tile([MEM, NT], f32, tag="colsum_ps", name=f"cs{c}")
            nc.tensor.matmul(cs[:], ones20_sb[:], pexp_t[c][:], start=True, stop=True)
            cs_t[c] = cs

        def recip_stage(c):
            # 1/colsum at ~18 correct bits
            rc = spool.tile([MEM, NT], f32, tag="recip", name=f"rc{c}")
            nc.vector.reciprocal_approx_fast(out=rc[:], in_=cs_t.pop(c)[:])
            pn_t[c] = (rc,)

        def u_stage(c):
            b, j = bj(c)
            o_sb = o_tiles[b]
            js = slice(j * NT, (j + 1) * NT)
            (rc,) = pn_t.pop(c)
            pn = spool.tile([MEM, NT], bf16, tag="pn", name=f"pn{c}")
            nc.vector.tensor_mul(pn[:], pexp_t.pop(c)[:], rc[:])
            for h in range(2):
                us = ppu.tile([128, NT], f32, tag="u_ps", name=f"us{c}_{h}")
                nc.tensor.matmul(us[:], qa_sb[:, b, h * 128:(h + 1) * 128], pn[:],
                                 start=True, stop=True)
                if h == 0:
                    nc.scalar.copy(o_sb[:, h, js], us[:])
                else:
                    nc.vector.tensor_copy(o_sb[:, h, js], us[:])

        def o_flush(c):
            # one store per chunk: [128, 4, NT] -> out channels (hh*128+p)
            b, j = bj(c)
            o_sb = o_tiles[b]
            js = slice(j * NT, (j + 1) * NT)
            dst = out_ext[b].rearrange("(hh p) n -> p hh n", p=128)
            nc.scalar.dma_start(dst[:, :, js], o_sb[:, :, js])

        # Chunk-level software pipeline: stage s of chunk c is emitted at
        # iteration c+s, so every cross-engine hop has a full iteration of
        # slack and the PE stream never waits on the softmax chain.
        for t in range(T + 6):
            if t < T:
                proj_chunk(t)
            if t == 0:
                load_attn_consts()
            if 0 <= t - 1 < T:
                logit_stage(t - 1)
            if 0 <= t - 2 < T:
                sum_stage(t - 2)
            if 0 <= t - 3 < T:
                recip_stage(t - 3)
            if 0 <= t - 4 < T:
                u_stage(t - 4)
            if 0 <= t - 5 < T:
                o_flush(t - 5)

    nc.compile()
    return nc


def kernel(feats, preds, labels, flag, queue, Wp, bp):
    from concourse.bass_utils import run_bass_kernel_spmd
    global LAST_RESULTS

    feats = np.ascontiguousarray(np.asarray(feats, dtype=np.float32))
    preds = np.ascontiguousarray(np.asarray(preds, dtype=np.float32))
    labels = np.asarray(labels).astype(np.int64)
    queue = np.ascontiguousarray(np.asarray(queue, dtype=np.float32))
    Wp = np.ascontiguousarray(np.asarray(Wp, dtype=np.float32))
    bp = np.ascontiguousarray(np.asarray(bp, dtype=np.float32))
    try:
        flag_v = int(np.asarray(flag))
    except TypeError:
        flag_v = int(flag)

    qfin = _host_queue_update(feats, preds, labels, flag_v, queue, Wp, bp)
    qA = np.ascontiguousarray(qfin[labels]).astype(BF16)         # [B, 20, 256]
    qAT = np.ascontiguousarray(qA.transpose(0, 2, 1))            # [B, 256, 20]
    wpt = np.ascontiguousarray(Wp.T).astype(BF16)                # [512, 256]
    bpc = np.ascontiguousarray(bp.reshape(2, 128).T)
    ones20 = np.ones((MEM, MEM), dtype=BF16)

    if "prog" not in _PROGRAM_CACHE:
        _PROGRAM_CACHE["prog"] = _build_program()
    nc = _PROGRAM_CACHE["prog"]

    f4 = feats.reshape(B, IN_C, HW).astype(BF16)
    in_maps = []
    for k in range(NCORES):
        s = slice(k * BPC, (k + 1) * BPC)
        in_maps.append({
            "feats": np.ascontiguousarray(f4[s]),
            "wpt": wpt,
            "bpc": bpc,
            "qat": np.ascontiguousarray(qAT[s]),
            "qa": np.ascontiguousarray(qA[s]),
            "ones20": ones20,
        })

    trace = bool(int(os.environ.get("KERNEL_TRACE", "0")))
    tc_env = os.environ.get("KERNEL_TRACE_CORES", "")
    trace_cores = [int(x) for x in tc_env.split(",") if x] or None
    res = run_bass_kernel_spmd(nc, in_maps, core_ids=list(range(NCORES)),
                               trace=trace, trace_cores=trace_cores)
    LAST_RESULTS = res
    out = np.concatenate([np.asarray(res.results[k]["out"]).astype(np.float32)
                          for k in range(NCORES)], axis=0)
    return out.reshape(B, 2 * CODE, H, W_SP)


if __name__ == "__main__":
    d = np.load("/tmp/inputs.npz")
    out = kernel(d["feats"], d["preds"], d["labels"], d["flag"], d["queue"], d["Wp"], d["bp"])
    exp = np.load("/tmp/expected.npy")
    err = np.abs(out - exp)
    print("absmax err:", err.max(), "scale-rel:", err.max() / np.abs(exp).max())


# revision 3
# speedup vs baseline: 1.2078x; 1.2078x over previous
"""TRN2 Bass kernel for nn_ClassSemantic (scatter_memory).

Strategy
--------
Data-parallel over batch: core k owns samples 4k..4k+3 and runs
projection (1x1 conv) + memory-gather attention + concat.

The problem is HBM-bandwidth bound (feats in + concat out dominate), so
all device I/O is bf16: the host casts feats/weights/queue rows to bf16
before staging, the device computes with fp32 PSUM accumulation, writes
the output as bf16, and the host upcasts to fp32.  This halves HBM
traffic vs fp32 (33.5 MB/core -> ~94 us roofline at 358 GB/s).  Host
also pre-permutes feats and un-permutes the output so every DMA line is
4 KB contiguous per partition.

The sequential EMA queue update depends on the per-sample masked
feature means only, which are algebraically separable:
    feat_b = mean_hw((Wp@f + bp) * pred) = Wp @ mean_hw(f * pred) + bp * mean(pred)
The inner reduction is computed on the host, the 32-step EMA scan runs
on the host in float64, and the final queue rows ship to every core as
constants.  The device never needs a collective.

Attention per chunk of 512 spatial positions:
  logit MM uses a replicated lhsT [128, 84] (three copies of the 20
  queue rows at column offsets 0/32/64, zero-padded) so exp() lands the
  same 20xNT block at SBUF partitions 0/32/64.  Then colsum / u-half-0 /
  u-half-1 run as three ROW-TILED matmuls (32x128 array tiling, row
  groups 0/32/64) that execute concurrently on the PE.  colsum uses an
  all-ones [20,128] lhsT so every output partition carries the softmax
  denominator; normalization happens after the u matmul as
  u = us * recip(colsum) on DVE, so softmax costs only one PE slot plus
  3 ACT passes (2x proj-bias copy, exp) and 3 DVE passes (recip, 2x mul)
  per chunk.

PSUM budget (8 banks): proj [128,2,NT] x2bufs = 4, logit [84,NT] x1,
colsum [128,NT] x1, u [128,2,NT] x1 = 2.

Rings: feats loads + weight consts on the SP HWDGE ring (sync), stores
plus attention consts on the ACT HWDGE ring (scalar) -- stores can
stall on compute without blocking load prefetch.
"""
import os
import numpy as np
import ml_dtypes
from contextlib import ExitStack

B, IN_C, H, W_SP = 32, 512, 64, 64
CODE, CLASSES, MEM = 256, 4, 20
HW = H * W_SP              # 4096
NCORES = 8
BPC = B // NCORES          # 4 samples per core
DECAY, EPS = 0.9, 1e-12
NCH = 8                    # n-chunks per sample
NT = HW // NCH             # 512 spatial positions per chunk
M3 = 84                    # 3 row-tiles of 20 slots (+12 pad each)

BF16 = ml_dtypes.bfloat16

_PROGRAM_CACHE = {}
LAST_RESULTS = None        # stash for test harness introspection


def _host_queue_update(feats, preds, labels, flag, queue, Wp, bp):
    """Final queue after the reference's sequential EMA scan (float64)."""
    if int(flag) != 1:
        return queue.astype(np.float32)
    f3 = feats.reshape(B, IN_C, HW)
    p2 = preds.reshape(B, HW)
    g = np.matmul(f3, p2[:, :, None])[:, :, 0] / np.float32(HW)
    feat = g @ Wp.T.astype(np.float32) + bp[None, :] * p2.mean(axis=1)[:, None]
    q = queue.astype(np.float64)
    for i in range(B):
        l = int(labels[i])
        f = feat[i].astype(np.float64)
        slot = q[l]
        logit = slot @ f
        upd = logit[:, None] * f[None, :]
        nrm = np.sqrt((upd * upd).sum(axis=1, keepdims=True))
        upd = upd / np.maximum(nrm, EPS)
        q[l] = DECAY * slot + (1.0 - DECAY) * upd
    return q.astype(np.float32)


def _build_program():
    from concourse import bacc, mybir
    import concourse.tile as tile

    f32, bf16 = mybir.dt.float32, mybir.dt.bfloat16
    nc = bacc.Bacc("TRN2", target_bir_lowering=False, debug=False)

    feats_in = nc.dram_tensor("feats", [128, BPC, NCH, 4, NT], bf16,
                              kind="ExternalInput").ap()
    wpt_in = nc.dram_tensor("wpt", [IN_C, CODE], bf16, kind="ExternalInput").ap()
    bp_in = nc.dram_tensor("bpc", [128, 2], f32, kind="ExternalInput").ap()
    qat3_in = nc.dram_tensor("qat3", [BPC, IN_C // 2, M3], bf16,
                             kind="ExternalInput").ap()
    qtrio_in = nc.dram_tensor("qtrio", [M3, BPC, 128], bf16,
                              kind="ExternalInput").ap()
    out_ext = nc.dram_tensor("out", [128, BPC, NCH, 4, NT], bf16,
                             kind="ExternalOutput").ap()

    with tile.TileContext(nc) as tc, ExitStack() as ctx:
        consts = ctx.enter_context(tc.tile_pool(name="consts", bufs=1))
        fpool = ctx.enter_context(tc.tile_pool(name="fpool", bufs=4))
        opool = ctx.enter_context(tc.tile_pool(name="opool", bufs=7))
        epool = ctx.enter_context(tc.tile_pool(name="epool", bufs=3))
        rpool = ctx.enter_context(tc.tile_pool(name="rpool", bufs=2))
        ppp = ctx.enter_context(tc.tile_pool(name="ppp", bufs=2, space="PSUM"))
        ppl = ctx.enter_context(tc.tile_pool(name="ppl", bufs=1, space="PSUM"))
        ppc = ctx.enter_context(tc.tile_pool(name="ppc", bufs=1, space="PSUM"))
        ppu = ctx.enter_context(tc.tile_pool(name="ppu", bufs=1, space="PSUM"))

        # projection weights on the sync ring ahead of the first feats
        # chunk; attention consts on the scalar ring (needed one pipeline
        # stage later, hides behind the ACT table load)
        wpt_sb = consts.tile([128, 4, CODE], bf16, name="wpt_sb")       # [p, kk, o]
        nc.sync.dma_start(wpt_sb[:], wpt_in.rearrange("(kk p) m -> p kk m", p=128))
        bp_sb = consts.tile([128, 2], f32, name="bp_sb")                # [p, half]
        nc.sync.dma_start(bp_sb[:], bp_in[:])
        qat3_sb = consts.tile([128, BPC, 2, M3], bf16, name="qat3_sb")  # [p, b, kk, m]
        qtrio_sb = consts.tile([M3, BPC, 128], bf16, name="qtrio_sb")   # [m3, b, c]

        def load_attn_consts():
            nc.scalar.dma_start(qat3_sb[:], qat3_in.rearrange("b (kk p) m -> p b kk m", p=128))
            nc.scalar.dma_start(qtrio_sb[:], qtrio_in[:])

        o_tiles = {}
        pexp_t = {}
        cs_t = {}
        us_t = {}
        T = BPC * NCH

        def bj(c):
            return c // NCH, c % NCH

        def proj_stage(c):
            b, j = bj(c)
            # hh 0..1: attention half (channels 0..255); hh 2..3: x half
            o_t = opool.tile([128, 4, NT], bf16, tag="o_t", name=f"o_t{c}")
            o_tiles[c] = o_t
            ft = fpool.tile([128, 4, NT], bf16, tag="ft", name=f"ft{c}")
            nc.sync.dma_start(ft[:], feats_in[:, b, j])
            ps = ppp.tile([128, 2, NT], f32, tag="proj_ps", name=f"pps{c}")
            for h in range(2):
                for kk in range(4):
                    nc.tensor.matmul(
                        ps[:, h, :], wpt_sb[:, kk, h * 128:(h + 1) * 128], ft[:, kk, :],
                        start=(kk == 0), stop=(kk == 3))
                nc.scalar.activation(
                    o_t[:, 2 + h, :], ps[:, h, :],
                    mybir.ActivationFunctionType.Identity,
                    bias=bp_sb[:, h:h + 1])

        def logit_stage(c):
            b, j = bj(c)
            o_t = o_tiles[c]
            lg = ppl.tile([M3, NT], f32, tag="logit_ps", name=f"lg{c}")
            for kk in range(2):
                nc.tensor.matmul(lg[:], qat3_sb[:, b, kk, :], o_t[:, 2 + kk, :],
                                 start=(kk == 0), stop=(kk == 1))
            pexp = epool.tile([M3, NT], bf16, tag="pexp", name=f"pexp{c}")
            nc.scalar.activation(pexp[:], lg[:], mybir.ActivationFunctionType.Exp)
            pexp_t[c] = pexp

        def trio_stage(c):
            # three row-tiled matmuls (row groups 0/32/64) run concurrently:
            # colsum (all-ones lhsT -> denominator on all 128 partitions),
            # u half 0, u half 1.  tile_position is inferred from the
            # partition offsets of the lhsT/rhs slices.
            b, j = bj(c)
            pexp = pexp_t.pop(c)
            cs = ppc.tile([128, NT], f32, tag="colsum_ps", name=f"cs{c}")
            us = ppu.tile([128, 2, NT], f32, tag="u_ps", name=f"us{c}")
            nc.tensor.matmul(cs[:], qtrio_sb[0:20, b, :], pexp[0:20, :],
                             start=True, stop=True)
            nc.tensor.matmul(us[:, 0, :], qtrio_sb[32:52, b, :], pexp[32:52, :],
                             start=True, stop=True)
            nc.tensor.matmul(us[:, 1, :], qtrio_sb[64:84, b, :], pexp[64:84, :],
                             start=True, stop=True)
            cs_t[c] = cs
            us_t[c] = us

        def norm_stage(c):
            o_t = o_tiles[c]
            cs = cs_t.pop(c)
            us = us_t.pop(c)
            rc = rpool.tile([128, NT], f32, tag="recip", name=f"rc{c}")
            nc.vector.reciprocal_approx_fast(out=rc[:], in_=cs[:])
            nc.vector.tensor_mul(o_t[:, 0, :], us[:, 0, :], rc[:])
            nc.vector.tensor_mul(o_t[:, 1, :], us[:, 1, :], rc[:])

        def store_stage(c):
            b, j = bj(c)
            nc.scalar.dma_start(out_ext[:, b, j], o_tiles.pop(c)[:])

        for t in range(T + 4):
            if t < T:
                proj_stage(t)
            if t == 0:
                load_attn_consts()
            if 0 <= t - 1 < T:
                logit_stage(t - 1)
            if 0 <= t - 2 < T:
                trio_stage(t - 2)
            if 0 <= t - 3 < T:
                norm_stage(t - 3)
            if 0 <= t - 4 < T:
                store_stage(t - 4)

    nc.compile()
    return nc


def kernel(feats, preds, labels, flag, queue, Wp, bp):
    from concourse.bass_utils import run_bass_kernel_spmd
    global LAST_RESULTS

    feats = np.ascontiguousarray(np.asarray(feats, dtype=np.float32))
    preds = np.ascontiguousarray(np.asarray(preds, dtype=np.float32))
    labels = np.asarray(labels).astype(np.int64)
    queue = np.ascontiguousarray(np.asarray(queue, dtype=np.float32))
    Wp = np.ascontiguousarray(np.asarray(Wp, dtype=np.float32))
    bp = np.ascontiguousarray(np.asarray(bp, dtype=np.float32))
    try:
        flag_v = int(np.asarray(flag))
    except TypeError:
        flag_v = int(flag)

    qfin = _host_queue_update(feats, preds, labels, flag_v, queue, Wp, bp)
    qA = qfin[labels].astype(BF16)                               # [B, 20, 256]

    # logit lhsT: three replicas of the 20 queue rows at column offsets
    # 0/32/64 (zero-padded), transposed to [B, 256, 84]
    qat3 = np.zeros((B, CODE, M3), dtype=BF16)
    for r in range(3):
        qat3[:, :, 32 * r:32 * r + MEM] = qA.transpose(0, 2, 1)
    # trio lhsT: rows 0-19 all-ones (colsum), 32-51 u half 0, 64-83 u half 1
    qtrio = np.zeros((M3, B, 128), dtype=BF16)
    qtrio[0:MEM] = 1.0
    qtrio[32:52] = qA[:, :, 0:128].transpose(1, 0, 2)
    qtrio[64:84] = qA[:, :, 128:256].transpose(1, 0, 2)

    wpt = np.ascontiguousarray(Wp.T).astype(BF16)                # [512, 256]
    bpc = np.ascontiguousarray(bp.reshape(2, 128).T)

    if "prog" not in _PROGRAM_CACHE:
        _PROGRAM_CACHE["prog"] = _build_program()
    nc = _PROGRAM_CACHE["prog"]

    # per-core feats relayout: [BPC, (kk p), (j n)] -> [p, b, j, kk, n]
    # so every chunk load is one contiguous 4 KB line per partition
    fb = feats.reshape(B, IN_C, HW).astype(BF16)
    in_maps = []
    for k in range(NCORES):
        s = slice(k * BPC, (k + 1) * BPC)
        fre = np.ascontiguousarray(
            fb[s].reshape(BPC, 4, 128, NCH, NT).transpose(2, 0, 3, 1, 4))
        in_maps.append({
            "feats": fre,
            "wpt": wpt,
            "bpc": bpc,
            "qat3": np.ascontiguousarray(qat3[s]),
            "qtrio": np.ascontiguousarray(qtrio[:, s]),
        })

    trace = bool(int(os.environ.get("KERNEL_TRACE", "0")))
    tc_env = os.environ.get("KERNEL_TRACE_CORES", "")
    trace_cores = [int(x) for x in tc_env.split(",") if x] or None
    res = run_bass_kernel_spmd(nc, in_maps, core_ids=list(range(NCORES)),
                               trace=trace, trace_cores=trace_cores)
    LAST_RESULTS = res
    # device out [p, b, j, hh, n] -> [b, (hh p), (j n)]
    outs = []
    for k in range(NCORES):
        o = np.asarray(res.results[k]["out"])
        outs.append(o.transpose(1, 3, 0, 2, 4).astype(np.float32)
                    .reshape(BPC, 2 * CODE, HW))
    out = np.concatenate(outs, axis=0)
    return out.reshape(B, 2 * CODE, H, W_SP)


if __name__ == "__main__":
    d = np.load("/tmp/inputs.npz")
    out = kernel(d["feats"], d["preds"], d["labels"], d["flag"], d["queue"], d["Wp"], d["bp"])
    exp = np.load("/tmp/expected.npy")
    err = np.abs(out - exp)
    print("absmax err:", err.max(), "scale-rel:", err.max() / np.abs(exp).max())
